# revision 4
# baseline (speedup 1.0000x reference)
"""MoE GroupedExperts kernel for 8 TRN2 NeuronCores.

Expert-parallel: expert e's tokens + weights go to core e. Tokens are
pre-sorted by expert, so routing is host-side slicing. Each core runs a
SwiGLU MLP: o = (silu(x @ gate) * (x @ up)) @ down.

Device compute in fp16 (fp32 accumulation in PSUM). Weights are
host-swizzled into a chunk-major SBUF-image layout so every DMA chunk is
one contiguous run per partition (128 large descriptors per chunk
instead of ~1K small ones) -- the HWDGE descriptor generator and HBM
both run at line rate. A short burst of dummy matmuls at kernel start
warms the PE HAM clock gate during the initial DMA fill.
"""

import sys

if "/opt/trn_rl_repo" not in sys.path:
    sys.path.insert(0, "/opt/trn_rl_repo")

import numpy as np

BF16 = np.float16
E = 8
DIM = 1024
HID = 2048
N_CORES = 8
CMAX_BLOCK = 512  # max tokens per device invocation (PSUM free-dim limit)

KC = DIM // 128    # 8 k-chunks for gate/up contraction
KH = HID // 128    # 16 k-chunks for down contraction
NH = HID // 128    # 16 hid slices of the gate/up output
CH = 256           # gate/up weight DMA chunk width (hid cols)
NCH = HID // CH    # 8 chunks per gate/up matrix
DKG = 4            # down-proj weight DMA chunks (by k-range)

_cache = {}


def _build(cpad: int):
    """Build + compile the per-core kernel for cpad tokens per expert."""
    from concourse import bacc
    import concourse.tile as tile
    import concourse.mybir as mybir

    f32 = mybir.dt.float32
    bf16 = mybir.dt.float16  # fp16: same PE rate as bf16, 3 more mantissa bits

    NTOK = cpad // 128  # token tiles

    nc = bacc.Bacc("TRN2", target_bir_lowering=False, debug=False)
    # All inputs are host-pre-swizzled into SBUF-image layout: leading
    # axis is the partition, and each DMA chunk is contiguous per
    # partition in both DRAM and SBUF.
    xt_d = nc.dram_tensor("xt", [128, KC, cpad], bf16, kind="ExternalInput")
    gw_d = nc.dram_tensor("gw", [128, NCH, KC, CH], bf16, kind="ExternalInput")
    uw_d = nc.dram_tensor("uw", [128, NCH, KC, CH], bf16, kind="ExternalInput")
    dw_d = nc.dram_tensor("dw", [128, DKG, KH // DKG, DIM], bf16, kind="ExternalInput")
    o_d = nc.dram_tensor("o", [cpad, DIM], bf16, kind="ExternalOutput")

    # Pair hid slices so one PSUM bank (512 fp32/partition) holds a
    # whole silu/mul group -- fewer, larger ACT/DVE ops and fewer sems.
    PAIR = max(1, min(NH, 512 // cpad))
    NG = NH // PAIR  # hid groups (== NCH when PAIR*128 == CH)

    with tile.TileContext(nc) as tc:
        with (
            tc.tile_pool(name="sb", bufs=1) as sb,
            tc.tile_pool(name="stmp", bufs=2) as stmp_pool,
            tc.tile_pool(name="ht", bufs=NG) as ht_pool,
            tc.tile_pool(name="outp", bufs=2) as outp,
            tc.tile_pool(name="psA", bufs=2, space="PSUM") as psA,
            tc.tile_pool(name="psB", bufs=2, space="PSUM") as psB,
            tc.tile_pool(name="psO", bufs=4, space="PSUM") as psO,
        ):
            xt_s = sb.tile([128, KC, cpad], bf16)
            gw_s = sb.tile([128, NCH, KC, CH], bf16)
            uw_s = sb.tile([128, NCH, KC, CH], bf16)
            dw_s = sb.tile([128, DKG, KH // DKG, DIM], bf16)

            # PE warm-up: the HAM clock gate holds the PE at 1.2 GHz
            # until it has seen ~3.4us of sustained activity. Run dummy
            # matmuls during the initial DMA fill so the real matmuls
            # start at (or soon reach) 2.4 GHz. They read dw_s before
            # its DMA lands -- the values are garbage and the result is
            # discarded (warm PSUM bank is overwritten by the down
            # projection's start=True much later); the only dep this
            # creates is dw's DMA waiting for the warm reads, which
            # finish long before dw's descriptor turn on the ring.
            # Sized so the burst ends right as x + the first weight
            # chunk land (~10us): 9 matmuls x 427ns cold.
            warm_ps = psO.tile([128, 512], f32, tag="po", name="warm")
            for _ in range(9):
                nc.tensor.matmul(
                    warm_ps[:], dw_s[:, 0, 0, 0:128], dw_s[:, 0, 0, 0:512],
                    start=True, stop=True, skip_group_check=True,
                )

            # DMA order == consumption order (strict FIFO on the sync
            # HWDGE ring): x first, then per hid group gate chunk then
            # up chunk, then the down-proj chunks by k-range. Every
            # chunk is one contiguous run per partition.
            nc.sync.dma_start(xt_s[:], xt_d.ap())
            for cc in range(NCH):
                nc.sync.dma_start(gw_s[:, cc], gw_d.ap()[:, cc])
                nc.sync.dma_start(uw_s[:, cc], uw_d.ap()[:, cc])
            for kg in range(DKG):
                nc.sync.dma_start(dw_s[:, kg], dw_d.ap()[:, kg])

            # Gate/up grouped GEMMs; h produced in [hid, tok] layout,
            # PAIR hid slices per PSUM bank side by side.
            ht = []
            for g in range(NG):
                pg = psA.tile([128, PAIR, cpad], f32, tag="pg")
                pu = psB.tile([128, PAIR, cpad], f32, tag="pu")
                # gate for both j before up: consumption matches the
                # gate-chunk-then-up-chunk DMA arrival order.
                for j in range(PAIR):
                    c0 = (g * PAIR + j) * 128
                    cc, oc = c0 // CH, c0 % CH
                    for k in range(KC):
                        nc.tensor.matmul(
                            pg[:, j, :], gw_s[:, cc, k, oc:oc + 128], xt_s[:, k, :],
                            start=(k == 0), stop=(k == KC - 1),
                            skip_group_check=True,
                        )
                for j in range(PAIR):
                    c0 = (g * PAIR + j) * 128
                    cc, oc = c0 // CH, c0 % CH
                    for k in range(KC):
                        nc.tensor.matmul(
                            pu[:, j, :], uw_s[:, cc, k, oc:oc + 128], xt_s[:, k, :],
                            start=(k == 0), stop=(k == KC - 1),
                            skip_group_check=True,
                        )
                stmp = stmp_pool.tile([128, PAIR, cpad], f32, tag="stmp")
                nc.scalar.activation(
                    stmp[:], pg[:], mybir.ActivationFunctionType.Silu
                )
                ht_t = ht_pool.tile([128, PAIR, cpad], bf16, tag="ht")
                nc.vector.tensor_mul(ht_t[:], stmp[:], pu[:])
                ht.append(ht_t)

            # Down projection: o[tok, dim] = h @ down. Per 512-col PSUM
            # bank: accumulate over all KH chunks, then cast to fp16 and
            # DMA out immediately (double-buffered out tiles, output on
            # the scalar HWDGE ring so it never queues behind weights).
            NDC = DIM // 512
            for tok in range(NTOK):
                t0, t1 = tok * 128, (tok + 1) * 128
                po = [
                    psO.tile([128, 512], f32, tag="po", name=f"po{tok}_{dc}")
                    for dc in range(NDC)
                ]
                for kg in range(DKG):
                    for dc in range(NDC):
                        d0, d1 = dc * 512, (dc + 1) * 512
                        for ki in range(KH // DKG):
                            k = kg * (KH // DKG) + ki
                            nc.tensor.matmul(
                                po[dc][:],
                                ht[k // PAIR][:, k % PAIR, t0:t1],
                                dw_s[:, kg, ki, d0:d1],
                                start=(k == 0), stop=(k == KH - 1),
                                skip_group_check=True,
                            )
                for dc in range(NDC):
                    d0, d1 = dc * 512, (dc + 1) * 512
                    out_t = outp.tile([128, 512], bf16, tag="out")
                    nc.vector.tensor_copy(out_t[:], po[dc][:])
                    nc.scalar.dma_start(o_d[t0:t1, d0:d1], out_t[:])

    nc.compile()
    return nc


def _get_nc(cpad: int):
    if cpad not in _cache:
        _cache[cpad] = _build(cpad)
    return _cache[cpad]


def _swizzle_gu(w):
    # [DIM, HID] -> [p, cc, k, ch]: img[p, cc, k, ch] = w[k*128+p, cc*CH+ch]
    return np.ascontiguousarray(
        w.reshape(KC, 128, NCH, CH).transpose(1, 2, 0, 3)
    )


def _swizzle_dw(w):
    # [HID, DIM] -> [p, kg, ki, d]: img[p, kg, ki, d] = w[(kg*KI+ki)*128+p, d]
    KI = KH // DKG
    return np.ascontiguousarray(
        w.reshape(DKG, KI, 128, DIM).transpose(2, 0, 1, 3)
    )


def _run_block(nc, xt_blocks, weights, collect):
    """One SPMD invocation: xt_blocks[e] is [128, KC, cpad] fp16."""
    from concourse.bass_utils import run_bass_kernel_spmd

    in_maps = []
    for e in range(E):
        gw, uw, dw = weights[e]
        in_maps.append({"xt": xt_blocks[e], "gw": gw, "uw": uw, "dw": dw})
    kwargs = {} if collect is None else dict(collect.get("run_kwargs") or {})
    res = run_bass_kernel_spmd(nc, in_maps, core_ids=list(range(N_CORES)), **kwargs)
    if collect is not None:
        collect.setdefault("results", []).append(res)
    return [res.results[e]["o"] for e in range(E)]


def kernel(x, counts, gate_proj, up_proj, down_proj, _collect=None):
    x = np.ascontiguousarray(np.asarray(x, dtype=np.float32))
    counts = np.asarray(counts, dtype=np.int32)
    gate_proj = np.asarray(gate_proj, dtype=np.float32).astype(BF16)
    up_proj = np.asarray(up_proj, dtype=np.float32).astype(BF16)
    down_proj = np.asarray(down_proj, dtype=np.float32).astype(BF16)

    T = x.shape[0]
    offs = np.concatenate([[0], np.cumsum(counts)]).astype(np.int64)
    cmax = int(counts.max()) if counts.size else 128

    n_blocks = max(1, -(-cmax // CMAX_BLOCK))
    if n_blocks == 1:
        cpad = max(128, -(-cmax // 128) * 128)
    else:
        cpad = CMAX_BLOCK

    nc = _get_nc(cpad)
    weights = [
        (_swizzle_gu(gate_proj[e]), _swizzle_gu(up_proj[e]),
         _swizzle_dw(down_proj[e]))
        for e in range(E)
    ]

    out = np.empty((T, DIM), dtype=np.float32)  # o arrives fp16, upcast here
    for b in range(n_blocks):
        xt_blocks = []
        spans = []
        for e in range(E):
            c = int(counts[e])
            s0 = min(b * cpad, c)
            s1 = min((b + 1) * cpad, c)
            xe = x[offs[e] + s0:offs[e] + s1]
            if xe.shape[0] < cpad:
                xe = np.concatenate(
                    [xe, np.zeros((cpad - xe.shape[0], DIM), np.float32)], axis=0
                )
            # [cpad, DIM] -> [p, k, c]: img[p, k, c] = xe[c, k*128+p]
            xt = np.ascontiguousarray(
                xe.T.astype(BF16).reshape(KC, 128, cpad).transpose(1, 0, 2)
            )
            xt_blocks.append(xt)
            spans.append((s0, s1))
        outs = _run_block(nc, xt_blocks, weights, _collect)
        for e in range(E):
            s0, s1 = spans[e]
            if s1 > s0:
                out[offs[e] + s0:offs[e] + s1] = outs[e][: s1 - s0]
    return out
